# revision 12
# baseline (speedup 1.0000x reference)
"""DGLJTNNDecoder forward on 8 Trainium2 NeuronCores (Bass/Tile), v2.

Strategy (data-parallel over trees, 128 trees/core, weights replicated):
  Two independent 23-step GRU chains (forward / backward edges) per core,
  interleaved for latency hiding.  All GEMMs use fp8(e4m3) DoubleRow
  matmuls (2 k-chunks of 128 per instruction, 0.5 cyc/row) with weights
  pre-scaled by 64 on the host; the 1/64 is folded into the activation
  scale.  Chain GEMMs run in TRANSPOSED orientation (out = [feat, tree]),
  so the sigmoid/tanh activations write tiles that are directly the next
  GEMM's moving operand - no PE transposes, no copy-backs.

  The gathered per-(tree,node) embedding rows (emb_pre = [emb,1] @
  [Wz1|Wh1|Wr|U1] + biases, x64, computed on host) are DMA-streamed in
  transposed per-step tiles and added into PSUM via a single fp16
  identity matmul before the fp8 accumulation.

  Engine budget: ACT does sigmoid/tanh/relu/exp only; DVE does the GRU
  elementwise math (fp16, 4x mode) + fp8 casts; GpSimd does the p-head
  dot-product accumulations (SBUF only - no PSUM port); PE does GEMMs +
  psum inits.

  Phase A: chains + forward p blocks (act table: sigmoid/tanh/relu).
  Phase B: q head + backward p blocks (act table: exp/ln/relu).

  Per-core output: [qloss_sum, ploss_sum, qcnt, pcnt_delta] fp32; host
  combines across cores.
"""

import numpy as np
from contextlib import ExitStack

import ml_dtypes

import concourse.bass as bass
import concourse.bacc as bacc
import concourse.mybir as mybir
import concourse.tile as tile
from concourse.bass_utils import run_bass_kernel_spmd

F8 = mybir.dt.float8e4
F16 = mybir.dt.float16
F32 = mybir.dt.float32
AF = mybir.ActivationFunctionType
ALU = mybir.AluOpType
AX = mybir.AxisListType
DR = mybir.MatmulPerfMode.DoubleRow
NP8 = ml_dtypes.float8_e4m3fn

N_CORES = 8
T, L, H, LAT, V = 1024, 24, 450, 56, 780
TC = T // N_CORES          # 128 trees per core
NF = L - 1                 # 23 steps per chain
NE = 2 * NF                # 46 scan steps
WS = 64.0                  # weight scale folded into fp8 weights
IS = 1.0 / WS

# scan-step schedule (t = 0..45): src/dst nodes
SRC = [t for t in range(NF)] + [46 - t for t in range(NF, NE)]
DST = [t + 1 for t in range(NF)] + [45 - t for t in range(NF, NE)]
GZ_PREF = 5                # step-pair DMA prefetch depth
GZ_BUFS = 2 * (GZ_PREF + 2)


def _pad_kf(w):
    """[k<=512, f<=512] -> [512, 512] zero-padded fp32."""
    out = np.zeros((512, 512), np.float32)
    out[:w.shape[0], :w.shape[1]] = w
    return out


def _pack_w8(w):
    """[450,450] weight -> [128, 4(kc), 4(fc), 128] fp8 (x64, padded)."""
    p = _pad_kf(np.asarray(w, np.float32) * WS)
    p = p.reshape(4, 128, 4, 128).transpose(1, 0, 2, 3)   # [p, kc, fc, n]
    return np.ascontiguousarray(p.astype(NP8))


def _pack_u8(w, n):
    """[450,n] -> [128, 4(kc), n] fp8 (x64, k-padded)."""
    p = np.zeros((512, n), np.float32)
    p[:w.shape[0]] = np.asarray(w, np.float32) * WS
    p = p.reshape(4, 128, n).transpose(1, 0, 2)
    return np.ascontiguousarray(p.astype(NP8))


def _packT16(a):
    """[128(tree), 450] -> [128(p), 4(c), 128(tree)] fp16 transposed pad."""
    p = np.zeros((512, 128), np.float32)
    p[:a.shape[1]] = np.asarray(a, np.float32).T
    p = p.reshape(4, 128, 128).transpose(1, 0, 2)
    return np.ascontiguousarray(p.astype(np.float16))


def build_program():
    nc = bacc.Bacc("TRN2", target_bir_lowering=False, debug=False,
                   num_devices=N_CORES)

    din = {}
    def dram_in(name, shape, dtype):
        din[name] = nc.dram_tensor(name, list(shape), dtype,
                                   kind="ExternalInput").ap()
        return din[name]

    dram_in("gz", [NE, 128, 12, 128], F16)  # per-step [zT|hT|rT] x64
    dram_in("gp", [128, L, H], F16)         # p-head gathered rows x64
    dram_in("qmask", [128, L, V], F16)      # one-hot of wid targets
    dram_in("wz8", [128, 4, 4, 128], F8)
    dram_in("wh8", [128, 4, 4, 128], F8)
    dram_in("ur8", [128, 4, 4, 128], F8)
    dram_in("w18", [128, 4, 4, 128], F8)
    dram_in("u28", [128, 4, H], F8)
    dram_in("wo8", [128, 4, V], F8)
    dram_in("tvWT", [128, 4, 128], F16)     # 64*(tv@W4+W_b) transposed
    dram_in("us16", [128, H], F16)          # Us broadcast
    dram_in("ident", [128, 128], F16)
    dram_in("ones32", [128, 1], F32)
    out_d = nc.dram_tensor("out", [4, 1], F32, kind="ExternalOutput").ap()

    with tile.TileContext(nc) as tc, ExitStack() as ctx:
        _kern(ctx, tc, din, out_d)

    nc.compile()
    return nc


def _kern(ctx, tc, din, out_d):
    nc = tc.nc

    pc = ctx.enter_context(tc.tile_pool(name="const", bufs=1))
    pm8 = ctx.enter_context(tc.tile_pool(name="m8", bufs=1))    # mT8 x46
    pacc = ctx.enter_context(tc.tile_pool(name="acc", bufs=1))

    def const_tile(name, shape, dtype):
        t = pc.tile(list(shape), dtype, tag=name, name=name)
        nc.sync.dma_start(t[:], din[name][:])
        return t

    ident = const_tile("ident", [128, 128], F16)
    wz8 = const_tile("wz8", [128, 4, 4, 128], F8)
    wh8 = const_tile("wh8", [128, 4, 4, 128], F8)
    ur8 = const_tile("ur8", [128, 4, 4, 128], F8)
    u28 = const_tile("u28", [128, 4, H], F8)
    us16 = const_tile("us16", [128, H], F16)
    gp_t = const_tile("gp", [128, L, H], F16)

    # fp8 m tiles: forward mT8f[k] and backward mT8b[k] persist (q/p heads)
    mT8f = [pm8.tile([128, 4, 128], F8, tag=f"mT8f{k}", name=f"mT8f{k}")
            for k in range(NF)]
    mT8b = [pm8.tile([128, 4, 128], F8, tag=f"mT8b{k}", name=f"mT8b{k}")
            for k in range(NF)]
    mT8 = {"f": mT8f, "b": mT8b}

    # accumulators
    seq_buf = pacc.tile([128, L], F32, tag="seq")
    ltgt_buf = pacc.tile([128, L], F32, tag="ltgt")
    mx_buf = pacc.tile([128, L], F32, tag="mx")
    plbuf = pacc.tile([128, NE + 1], F32, tag="pl")

    w18 = const_tile("w18", [128, 4, 4, 128], F8)
    wo8 = const_tile("wo8", [128, 4, V], F8)
    tvWT = const_tile("tvWT", [128, 4, 128], F16)
    ones32 = const_tile("ones32", [128, 1], F32)

    def mm(ps, lhsT, rhs, start, stop, pm=None):
        nc.tensor.matmul(ps, lhsT, rhs, start=start, stop=stop,
                         perf_mode=pm)

    qm_t = pc.tile([128, L, V], F16, tag="mk", name="qm_t")

    # ---------------- phase A: chains + forward p blocks ----------------
    pgz_cm = tc.tile_pool(name="gz", bufs=1)
    pgz = pgz_cm.__enter__()
    gz_tiles = [pgz.tile([128, 12, 128], F16, tag=f"gz{i}", name=f"gz{i}")
                for i in range(GZ_BUFS)]

    def gz_dma(t):
        nc.sync.dma_start(gz_tiles[t % GZ_BUFS][:], din["gz"][t])

    for t in range(2 * GZ_PREF):
        gz_dma(t)

    with tc.tile_pool(name="chps", bufs=1, space="PSUM") as chps, \
         tc.tile_pool(name="chsb", bufs=2) as chsb:

        m16 = {}     # fp16 carry per chain
        carries = {}

        def chain_gemm(ps, gz_sec, w8, rhs8, first):
            """psum[128,4,128] = gz_sec (ident init) + rhs8 @ w8 (DR)."""
            for c in range(4):
                mm(ps[:, c, :], ident[:, :], gz_sec[:, c, :], True, first)
                if not first:
                    for P in range(2):
                        mm(ps[:, c, :], w8[:, 2 * P:2 * P + 2, c, :],
                           rhs8[:, 2 * P:2 * P + 2, :],
                           False, P == 1, pm=DR)

        def chain_step(k, ch):
            t_gz = 2 * k + (0 if ch == "f" else 1)
            first = (k == 0)
            last = (k == NF - 1)
            gz = gz_tiles[t_gz % GZ_BUFS]
            if t_gz + 2 * GZ_PREF < NE:
                gz_dma(t_gz + 2 * GZ_PREF)

            ps_h = chps.tile([128, 4, 128], F32, tag=f"h{ch}",
                             name=f"psh{ch}{k}")
            chain_gemm(ps_h, gz[:, 4:8, :], wh8,
                       None if first else carries[ch]["rm8"], first)
            ps_z = chps.tile([128, 4, 128], F32, tag=f"z{ch}",
                             name=f"psz{ch}{k}")
            chain_gemm(ps_z, gz[:, 0:4, :], wz8,
                       None if first else mT8[ch][k - 1], first)

            mt_t = chsb.tile([128, 4, 128], F16, tag=f"mt{ch}",
                             name=f"mt{ch}{k}")
            nc.scalar.activation(mt_t[:], ps_h[:], AF.Tanh, scale=IS)
            z_t = chsb.tile([128, 4, 128], F16, tag=f"zt{ch}",
                            name=f"zt{ch}{k}")
            nc.scalar.activation(z_t[:], ps_z[:], AF.Sigmoid, scale=IS)

            m_new = chsb.tile([128, 4, 128], F16, tag=f"mn{ch}{k % 2}",
                              name=f"mn{ch}{k}")
            if first:
                nc.vector.tensor_mul(m_new[:], z_t[:], mt_t[:])
            else:
                s_t = m16[ch]
                d1 = chsb.tile([128, 4, 128], F16, tag=f"d1{ch}",
                               name=f"d1{ch}{k}")
                nc.vector.tensor_sub(d1[:], mt_t[:], s_t[:])
                d2 = chsb.tile([128, 4, 128], F16, tag=f"d2{ch}",
                               name=f"d2{ch}{k}")
                nc.vector.tensor_mul(d2[:], z_t[:], d1[:])
                nc.vector.tensor_add(m_new[:], s_t[:], d2[:])
            m16[ch] = m_new
            nc.vector.tensor_copy(mT8[ch][k][:], m_new[:])

            if last:
                return
            ps_r = chps.tile([128, 4, 128], F32, tag=f"r{ch}",
                             name=f"psr{ch}{k}")
            chain_gemm(ps_r, gz[:, 8:12, :], ur8, mT8[ch][k], False)
            r_t = chsb.tile([128, 4, 128], F16, tag=f"rt{ch}",
                            name=f"rt{ch}{k}")
            nc.scalar.activation(r_t[:], ps_r[:], AF.Sigmoid, scale=IS)
            rm8 = chsb.tile([128, 4, 128], F8, tag=f"rm8{ch}{k % 2}",
                            name=f"rm8{ch}{k}")
            nc.vector.tensor_mul(rm8[:], r_t[:], m_new[:])
            carries[ch] = {"rm8": rm8}

        def p_block(j):
            """p logit block j: us . relu((gp[node] + hs@U2)/64)."""
            node = 0 if j == 0 else DST[j - 1]
            ps_p = chps.tile([128, H], F32, tag="pa", name=f"psp{j}")
            mm(ps_p[:], ident[:, :], gp_t[:, node, :], True, j == 0)
            if j > 0:
                hT8 = mT8f[j - 1]
                for P in range(2):
                    mm(ps_p[:], hT8[:, 2 * P:2 * P + 2, :],
                       u28[:, 2 * P:2 * P + 2, :], False, P == 1, pm=DR)
            pa = chsb.tile([128, H], F16, tag="pa", name=f"pa{j}")
            nc.scalar.activation(pa[:], ps_p[:], AF.Relu, scale=IS)
            pt_s = chsb.tile([128, H], F16, tag="pts", name=f"pts{j}")
            nc.vector.scalar_tensor_tensor(
                pt_s[:], pa[:], 1.0, us16[:], op0=ALU.mult,
                op1=ALU.mult, accum_out=plbuf[:, j:j + 1])

        p_block(0)
        for k in range(NF):
            chain_step(k, "f")
            chain_step(k, "b")
            p_block(k + 1)
            if k % 6 == 2:           # stream qmask in chunks mid-phase-A
                q0 = (k // 6) * 6
                nc.sync.dma_start(qm_t[:, q0:q0 + 6, :],
                                  din["qmask"][:, q0:q0 + 6, :])

    pgz_cm.__exit__(None, None, None)

    # ---------------- phase B: q head + backward p blocks ----------------
    with tc.tile_pool(name="hps", bufs=1, space="PSUM") as hps, \
         tc.tile_pool(name="hsb", bufs=2) as hsb:

        qaT8 = [pc.tile([128, 4, 128], F8, tag=f"qaT8{i}", name=f"qaT8{i}")
                for i in range(2)]
        for i in range(2):
            nc.vector.memset(qaT8[i][64:128, 3, :], 1.0)

        def p_block_b(j):
            """backward p block j (scan step t = j-1): hs = m_b + m_fwd."""
            kb = j - 1 - NF
            node = DST[j - 1]
            ifw = DST[j - 1] - 1              # forward edge into node
            hs8 = hsb.tile([128, 4, 128], F8, tag="hs8", name=f"hs8{j}")
            if ifw >= 0:
                nc.vector.tensor_add(hs8[:], mT8b[kb][:], mT8f[ifw][:])
                hT8 = hs8
            else:
                hT8 = mT8b[kb]
            ps_p = hps.tile([128, H], F32, tag="pb", bufs=1, name=f"psp{j}")
            mm(ps_p[:], ident[:, :], gp_t[:, node, :], True, False)
            for P in range(2):
                mm(ps_p[:], hT8[:, 2 * P:2 * P + 2, :],
                   u28[:, 2 * P:2 * P + 2, :], False, P == 1, pm=DR)
            pa = hsb.tile([128, H], F16, tag="pab", name=f"pab{j}")
            nc.scalar.activation(pa[:], ps_p[:], AF.Relu, scale=IS)
            pt_s = hsb.tile([128, H], F16, tag="ptsb", name=f"ptsb{j}")
            nc.vector.scalar_tensor_tensor(
                pt_s[:], pa[:], 1.0, us16[:], op0=ALU.mult,
                op1=ALU.mult, accum_out=plbuf[:, j:j + 1])

        for j in range(L):
            # q block j: qaT = relu((tvWT + W1^T @ hsT)/64), fp8 out
            ps_qa = hps.tile([128, 4, 128], F32, tag="qa", bufs=2,
                             name=f"psqa{j}")
            h8 = mT8f[j - 1] if j > 0 else None
            for c in range(4):
                mm(ps_qa[:, c, :], ident[:, :], tvWT[:, c, :], True, j == 0)
                if j > 0:
                    for P in range(2):
                        mm(ps_qa[:, c, :], w18[:, 2 * P:2 * P + 2, c, :],
                           h8[:, 2 * P:2 * P + 2, :],
                           False, P == 1, pm=DR)
            qa8 = qaT8[j % 2]
            nc.scalar.activation(qa8[:, 0:3, :], ps_qa[:, 0:3, :],
                                 AF.Relu, scale=IS)
            nc.scalar.activation(qa8[0:66, 3, :], ps_qa[0:66, 3, :],
                                 AF.Relu, scale=IS)

            ps_log = hps.tile([128, 1024], F32, tag="log", bufs=2,
                              name=f"pslog{j}")
            for v0, v1 in ((0, 512), (512, 780)):
                for P in range(2):
                    mm(ps_log[:, v0:v1], qa8[:, 2 * P:2 * P + 2, :],
                       wo8[:, 2 * P:2 * P + 2, v0:v1],
                       P == 0, P == 1, pm=DR)

            nc.vector.tensor_reduce(mx_buf[:, j:j + 1], ps_log[:, 0:V],
                                    axis=AX.X, op=ALU.max)
            exp_s = hsb.tile([128, V], F16, tag="exps", name=f"exps{j}")
            nc.scalar.activation(exp_s[:], ps_log[:, 0:V], AF.Exp,
                                 scale=IS, accum_out=seq_buf[:, j:j + 1])
            ttr_s = hsb.tile([128, V], F16, tag="ttrs", name=f"ttrs{j}")
            nc.vector.scalar_tensor_tensor(
                ttr_s[:], ps_log[:, 0:V], 1.0, qm_t[:, j, :], op0=ALU.mult,
                op1=ALU.mult, accum_out=ltgt_buf[:, j:j + 1])

            if j + NF + 1 <= NE:
                p_block_b(j + NF + 1)

        # ---- tails ----
        partials = pacc.tile([128, 4], F32, tag="partials")
        lnseq = hsb.tile([128, L], F32, tag="lnseq")
        nc.scalar.activation(lnseq[:], seq_buf[:], AF.Ln)
        qcol = hsb.tile([128, L], F32, tag="qcol")
        nc.vector.scalar_tensor_tensor(
            qcol[:], ltgt_buf[:], -IS, lnseq[:], op0=ALU.mult, op1=ALU.add)
        nc.vector.tensor_reduce(partials[:, 0:1], qcol[:], axis=AX.X,
                                op=ALU.add)
        qeq = hsb.tile([128, L], F32, tag="qeq")
        nc.vector.tensor_tensor(qeq[:], ltgt_buf[:], mx_buf[:],
                                ALU.is_equal)
        nc.vector.tensor_reduce(partials[:, 2:3], qeq[:], axis=AX.X,
                                op=ALU.add)

        # p loss: softplus(-pl) for blocks 0..22, softplus(pl) for 23..46
        e1 = hsb.tile([128, 23], F32, tag="e1")
        nc.scalar.activation(e1[:], plbuf[:, 0:23], AF.Exp, scale=-1.0)
        l1 = hsb.tile([128, 23], F32, tag="l1")
        nc.scalar.activation(l1[:], e1[:], AF.Ln, bias=ones32[:, 0:1])
        e0 = hsb.tile([128, 24], F32, tag="e0")
        nc.scalar.activation(e0[:], plbuf[:, 23:47], AF.Exp)
        l0 = hsb.tile([128, 24], F32, tag="l0")
        nc.scalar.activation(l0[:], e0[:], AF.Ln, bias=ones32[:, 0:1])
        sp1 = hsb.tile([128, 1], F32, tag="sp1")
        nc.vector.tensor_reduce(sp1[:], l1[:], axis=AX.X, op=ALU.add)
        sp0 = hsb.tile([128, 1], F32, tag="sp0")
        nc.vector.tensor_reduce(sp0[:], l0[:], axis=AX.X, op=ALU.add)
        nc.vector.tensor_add(partials[:, 1:2], sp1[:], sp0[:])

        gt = hsb.tile([128, NE + 1], F32, tag="gt")
        nc.vector.tensor_scalar(gt[:], plbuf[:], 0.0, None, op0=ALU.is_gt)
        s1 = hsb.tile([128, 1], F32, tag="s1")
        nc.vector.tensor_reduce(s1[:], gt[:, 0:23], axis=AX.X, op=ALU.add)
        s0 = hsb.tile([128, 1], F32, tag="s0")
        nc.vector.tensor_reduce(s0[:], gt[:, 23:47], axis=AX.X, op=ALU.add)
        nc.vector.tensor_sub(partials[:, 3:4], s1[:], s0[:])

        ps_out = hps.tile([4, 1], F32, tag="out", bufs=1)
        nc.tensor.matmul(ps_out[:], partials[:, :], ones32[:, :],
                         start=True, stop=True)
        out_sb = hsb.tile([4, 1], F32, tag="outsb")
        nc.scalar.copy(out_sb[:], ps_out[:])
        nc.sync.dma_start(out_d[:], out_sb[:])


# ------------------------------------------------------------------
_PROGRAM = None

def _get_program():
    global _PROGRAM
    if _PROGRAM is None:
        _PROGRAM = build_program()
    return _PROGRAM


def make_in_maps(wid, tree_vec, emb, W_w, W_b, U_w, U_b, Wo_w, Wo_b, Us_w,
                 Us_b, Wz_w, Wz_b, Wr_w, Ur_w, Ur_b, Wh_w, Wh_b):
    """Host-side shard + pack. Returns list of 8 per-core input dicts."""
    f32 = np.float32
    wid = np.asarray(wid); emb = np.asarray(emb, f32)
    tree_vec = np.asarray(tree_vec, f32)
    W_w, W_b = np.asarray(W_w, f32), np.asarray(W_b, f32)
    U_w, U_b = np.asarray(U_w, f32), np.asarray(U_b, f32)
    Wz_w, Wz_b = np.asarray(Wz_w, f32), np.asarray(Wz_b, f32)
    Wh_w, Wh_b = np.asarray(Wh_w, f32), np.asarray(Wh_b, f32)
    Wr_w = np.asarray(Wr_w, f32)
    Ur_w, Ur_b = np.asarray(Ur_w, f32), np.asarray(Ur_b, f32)

    # emb_pre over the vocab: ([emb,1] @ [Wz1|Wh1|Wr|U1] + biases) * 64
    wpre = np.concatenate([Wz_w[:H], Wh_w[:H], Wr_w, U_w[:H]], 1)
    bias_row = np.concatenate([Wz_b, Wh_b, Ur_b, np.zeros(H, f32)])
    emb_pre = (emb @ wpre + bias_row[None, :]) * WS        # [780, 1800]

    tvU = (tree_vec @ U_w[2 * H:] + U_b) * WS              # [T, 450]
    tvW = (tree_vec @ W_w[H:H + LAT] + W_b) * WS           # [T, 450]

    wo8 = _pack_u8(Wo_w, V)
    wo8[66, 3, :] = (Wo_b * WS).astype(NP8)

    shared = {
        "wz8": _pack_w8(Wz_w[H:]),
        "wh8": _pack_w8(Wh_w[H:]),
        "ur8": _pack_w8(Ur_w),
        "w18": _pack_w8(W_w[:H]),
        "u28": _pack_u8(U_w[H:2 * H], H),
        "wo8": wo8,
        "us16": np.ascontiguousarray(np.broadcast_to(
            np.asarray(Us_w, f32)[:, 0][None, :], (128, H))).astype(np.float16),
        "ident": np.eye(128, dtype=np.float16),
        "ones32": np.ones((128, 1), f32),
    }
    us_b = float(np.asarray(Us_b, f32)[0])
    assert us_b == 0.0, "Us_b folded as stt initial=0; nonzero needs plumb"

    in_maps = []
    for core in range(N_CORES):
        t0 = core * TC
        wc = wid[t0:t0 + TC]                       # [128, L]
        g = emb_pre[wc]                            # [128, L, 1800]
        gz = np.zeros((NE, 128, 12, 128), np.float16)
        for k in range(NF):
            for ci, tsc in ((0, k), (1, NF + k)):
                tg = 2 * k + ci
                src, dst = SRC[tsc], DST[tsc]
                gz[tg, :, 0:4, :] = _packT16(g[:, src, 0:450])
                gz[tg, :, 4:8, :] = _packT16(g[:, src, 450:900])
                gz[tg, :, 8:12, :] = _packT16(g[:, dst, 900:1350])
        gp = (g[:, :, 1350:1800] + tvU[t0:t0 + TC][:, None, :])
        qm = np.zeros((TC, L, V), np.float16)
        tt, jj = np.meshgrid(np.arange(TC), np.arange(L), indexing="ij")
        qm[tt, jj, wc] = 1.0
        m = dict(shared)
        m["gz"] = gz
        m["gp"] = np.ascontiguousarray(gp.astype(np.float16))
        m["qmask"] = qm
        m["tvWT"] = _packT16(tvW[t0:t0 + TC])
        in_maps.append(m)
    return in_maps


def combine(outs):
    """outs: list of 8 [4,1] arrays -> reference 4-tuple."""
    s = np.sum([np.asarray(o)[:, 0].astype(np.float64) for o in outs], axis=0)
    q_loss = np.float32(s[0] / T)
    p_loss = np.float32(s[1] / T)
    q_acc = np.float32(np.float32(s[2]) / np.float32(L * T))
    p_cnt = s[3] + N_CORES * 24 * TC
    p_acc = np.float32(np.float32(p_cnt) / np.float32((NE + 1) * T))
    return (q_loss, p_loss, q_acc, p_acc)


def run_on_cores(in_maps, trace=False, **kw):
    nc = _get_program()
    return run_bass_kernel_spmd(nc, in_maps, list(range(N_CORES)),
                                trace=trace, **kw)


def kernel(**inputs):
    in_maps = make_in_maps(**inputs)
    res = run_on_cores(in_maps)
    return combine([res.results[c]["out"] for c in range(N_CORES)])
